# revision 19
# baseline (speedup 1.0000x reference)
"""Trainium2 Bass kernel for multi-head attention (B=4, S=2048, D=1024, H=16).

Sharding: tensor-parallel over heads. 8 cores x 2 heads each.
Each core receives the full (transposed, bf16) q/k/v and its own head-slice
of the projection weights; it computes its heads' attention and writes an
unnormalized output [h, b, 65, S] where row 64 is the softmax denominator.
Host divides and reassembles.

Schedule (v2 — restructured fill/steady/tail vs the first version):
 - Per-tensor DMA queues: k on SP(sync), q0 split over DVE/Act/SP queues,
   batch>=1 q and all v on the GpSimd queue, outputs on GpSimd. The Act and
   DVE queues carry no steady-state DMA, so exp consumers never stall
   behind descriptor issue.
 - Batch-0 streams land column-block-major (4x8 small DMAs per tensor) so
   the first projection group's inputs arrive ~2x earlier; the first score
   step fires at ~14us instead of ~25us.
 - PE warm-up runs against a memset tile (no DMA dependency), starting the
   p-state ramp at ~6.5us.
 - qhT/khT/vh are 2-batch ring buffers, paying for per-tensor stream pools
   (k/q/v x 8 bufs) whose slot-reuse WAR waits are schedule-guaranteed to
   be no-ops (no descriptor-queue stalls).
 - Projection work drips between score steps gated by a per-queue wire
   model, with hard deadline caps so a mis-estimated gate can only stall
   the PE, never reorder past a consumer.
 - The last window runs its j1 PV pass in-window (lag-1 on odd kt, po in
   the ppsum pool) so only one PV emit + copies trail the final score.

Per-core steady state: windows of (batch, 1024 q-cols) proceed in 16
kt-steps; each step emits 4 score matmuls into four single-bank [128,512]
PSUM chunks, 4 exps split between Act table-exp and DVE Schraudolph PWL
(int16 bf16-bit-patterns, f=0.5 uniform), and 4 PV matmuls (one pass of 2
po tiles [65,512] at a time, row 64 = softmax denominator via a ones
column in vh).

Math notes:
 - attention_mask is all-False in the problem spec -> no-op; biases zero.
 - 1/sqrt(d_head) folded into Wq on the host.
 - softmax without max-subtraction: scores ~ N(0,1), exp safe in fp32.
 - PWL exp on 1/2 of tiles adds ~1.2e-2 rel err (validated on HW:
   1.36e-2 total), under the 2e-2 gate with ~30% margin.
"""

import os
import sys

import numpy as np

try:
    import concourse.bass as bass
except ImportError:
    sys.path.insert(0, "/opt/trn_rl_repo")
    import concourse.bass as bass

import ml_dtypes
from collections import deque
from contextlib import ExitStack

import concourse.tile as tile
from concourse import bacc, mybir
from concourse import bass_utils

BF16 = mybir.dt.bfloat16
F32 = mybir.dt.float32
I16 = mybir.dt.int16

# Problem sizes (hardcoded per spec)
B = 4
S = 2048
D = 1024
H = 16
DH = 64
N_CORES = 8
HL = H // N_CORES  # heads per core = 2

# PWL exp: int16 = round(x * 128*log2(e) + (127*128 - c)); bits = bf16(~e^x)
PWL_A = 128.0 * 1.4426950408889634
PWL_B = 127.0 * 128.0 - 7.33


def build_attention_nc(b=B, s=S, d=D, hl=HL, num_devices=N_CORES):
    """Build the per-core Bass graph. Same graph on all cores (SPMD)."""
    P = 128  # partitions
    KT = d // P          # contraction tiles for projections = 8
    ST = s // P          # sk tiles per sequence = 16
    NB = s // 512        # 512-wide blocks per sequence = 4
    NW = s // 1024       # windows per batch = 2
    FW = hl * DH         # feature width this core computes (= 128)
    assert FW == 128 and s % 1024 == 0

    nc = bacc.Bacc(
        "TRN2",
        target_bir_lowering=False,
        debug=False,
        num_devices=num_devices,
    )

    # host ships q/k/v chunk-major: [b, half, kt, p, 1024] so every stream
    # DMA chunk is a 256KB contiguous DRAM read (128KB for 512-col subreads)
    qT = nc.dram_tensor("qT", [b, 2, d // 128, 128, 1024], BF16,
                        kind="ExternalInput").ap()
    kTd = nc.dram_tensor("kT", [b, 2, d // 128, 128, 1024], BF16,
                         kind="ExternalInput").ap()
    vT = nc.dram_tensor("vT", [b, 2, d // 128, 128, 1024], BF16,
                        kind="ExternalInput").ap()
    # weights arrive host-permuted as [p, kt*FW] so one DMA loads each
    wq = nc.dram_tensor("wq", [P, KT * FW], BF16, kind="ExternalInput").ap()
    wk = nc.dram_tensor("wk", [P, KT * FW], BF16, kind="ExternalInput").ap()
    wv = nc.dram_tensor("wv", [P, KT * FW], BF16, kind="ExternalInput").ap()
    # output as contiguous [65, 512] chunks (chunk = sqh*2+j) so each out DMA
    # is a single large-run transfer: fast completion posting, cheap drain
    out = nc.dram_tensor(
        "out", [hl, b, (s // 1024) * 2, DH + 1, 512], F32, kind="ExternalOutput"
    ).ap()

    with tile.TileContext(nc) as tc, ExitStack() as ctx:
        persist = ctx.enter_context(tc.tile_pool(name="persist", bufs=1))
        kpool = ctx.enter_context(tc.tile_pool(name="kpool", bufs=KT))
        qpool = ctx.enter_context(tc.tile_pool(name="qpool", bufs=KT))
        vpool = ctx.enter_context(tc.tile_pool(name="vpool", bufs=KT))
        spsum = ctx.enter_context(tc.tile_pool(name="spsum", bufs=4, space="PSUM"))
        ppsum = ctx.enter_context(tc.tile_pool(name="ppsum", bufs=2, space="PSUM"))
        vpsum = ctx.enter_context(tc.tile_pool(name="vpsum", bufs=2, space="PSUM"))
        epool = ctx.enter_context(tc.tile_pool(name="epool", bufs=64))
        outpool = ctx.enter_context(tc.tile_pool(name="outpool", bufs=4))

        # weights in SBUF: [128, KT*128], k-tile kt at cols kt*128:(kt+1)*128
        wq_sb = persist.tile([P, KT * FW], BF16, tag="wq_sb")
        wk_sb = persist.tile([P, KT * FW], BF16, tag="wk_sb")
        wv_sb = persist.tile([P, KT * FW], BF16, tag="wv_sb")

        # projected activations: 2-batch ring buffers
        qhT_sb = persist.tile([P, 2 * s], BF16, tag="qhT_sb")
        khT_sb = persist.tile([P, 2 * s], BF16, tag="khT_sb")
        # vh ring: per (h, bi%2, st): [128, 65] block, col 64 == 1.0
        vh_sb = persist.tile([P, hl * 2 * ST, DH + 1], BF16, tag="vh_sb")
        # warm-up scratch (memset, no DMA dependency)
        warm = persist.tile([P, 640], BF16, tag="warm")

        def vidx(h, bi, st):
            return (h * 2 + bi % 2) * ST + st

        # ---------------- wire model (per DMA queue, in us) ----------------
        QSTART = 6.3     # engines free after framework preamble
        DESC_US = 0.68   # per-descriptor issue cost on the queue engine
        WIRE_BPUS = 220e3  # bytes per us per queue
        FILL_US = 12.0   # est. wall time of global step 0 (conservative low)
        STEP_US = 1.9    # optimistic step period (conservative for gating)
        wires = {q: [QSTART, 0] for q in ("sync", "vec", "act", "gps")}

        def wadd(q, nbytes, ndesc=1):
            w = wires[q]
            w[1] += ndesc
            w[0] = max(w[0] + nbytes / WIRE_BPUS,
                       QSTART + DESC_US * w[1] + nbytes / WIRE_BPUS / max(ndesc, 1))
            return w[0]

        def rg(t_us, margin=1):
            return int(np.ceil((t_us - FILL_US) / STEP_US)) + margin

        ENG = {"sync": None, "vec": None, "act": None, "gps": None}

        def qeng(q):
            return {"sync": nc.sync, "vec": nc.vector, "act": nc.scalar,
                    "gps": nc.gpsimd}[q]

        # ---------------- compute groups ----------------
        def qk_group(bi, xs_t, w_sb, dst, blk):
            """One q/k projection block group: 8 MMs + 1 cast copy."""
            ps = ppsum.tile([P, 512], F32, name="projp", tag="pp")
            for kt in range(KT):
                nc.tensor.matmul(
                    ps[:],
                    w_sb[:, kt * FW : (kt + 1) * FW],
                    xs_t[kt][:, blk * 512 : (blk + 1) * 512],
                    start=(kt == 0),
                    stop=(kt == KT - 1),
                )
            base = (bi % 2) * s
            nc.scalar.copy(dst[:, base + blk * 512 : base + (blk + 1) * 512], ps[:])

        def v_group(bi, xs_t, st):
            """One v projection st group: 8 MMs + 2 head copies into vh."""
            pv = ppsum.tile([P, FW], F32, name="vproj", tag="pp")
            for kt in range(KT):
                nc.tensor.matmul(
                    pv[:],
                    xs_t[kt][:, st * P : (st + 1) * P],
                    wv_sb[:, kt * FW : (kt + 1) * FW],
                    start=(kt == 0),
                    stop=(kt == KT - 1),
                )
            for h in range(hl):
                nc.vector.tensor_copy(
                    vh_sb[:, vidx(h, bi, st), 0:DH], pv[:, h * DH : (h + 1) * DH]
                )

        def et_rhs(ent):
            t, is_i16 = ent
            return t[:].bitcast(BF16) if is_i16 else t[:]

        def score_step(w, kt, ets):
            """4 score chunk MMs (head pairs pack) + 4 per-chunk exps."""
            bi, sqh = w
            q0 = (bi % 2) * s + sqh * 1024
            k0 = (bi % 2) * s + kt * P
            cs = {}
            for j in range(2):
                for h in range(hl):
                    hp = h * DH
                    c = spsum.tile([P, 512], F32, name=f"c{h}{j}", tag="sc")
                    nc.tensor.matmul(
                        c[:],
                        khT_sb[hp : hp + DH, k0 : k0 + P],
                        qhT_sb[hp : hp + DH, q0 + j * 512 : q0 + (j + 1) * 512],
                        start=True,
                        stop=True,
                    )
                    cs[(h, j)] = c
            for j in range(2):
                for h in range(hl):
                    # alternate act/DVE per chunk: f=0.5 per head, uniform
                    use_dve = (kt + h + j) % 2 == 0
                    if use_dve:
                        t = epool.tile([P, 512], I16, name="etd", tag="et")
                        nc.vector.tensor_scalar(
                            t[:], cs[(h, j)][:], PWL_A, PWL_B,
                            mybir.AluOpType.mult, mybir.AluOpType.add,
                        )
                        ets[h][j].append((t, True))
                    else:
                        t = epool.tile([P, 512], BF16, name="eta", tag="et")
                        nc.scalar.activation(
                            t[:], cs[(h, j)][:], mybir.ActivationFunctionType.Exp
                        )
                        ets[h][j].append((t, False))

        class PVPass:
            """One PV pass (w, j): 2 po tiles accumulated over 16 kk."""

            def __init__(self, w, ets, j, pool, final=False):
                self.w = w
                self.ets = ets
                self.j = j
                self.pool = pool
                self.final = final
                self.po = None

            def emit(self, local):
                """local in 0..7 -> kk = 2*local, 2*local+1 (4 MMs)."""
                if local == 0:
                    self.po = [
                        self.pool.tile([DH + 1, 512], F32, name=f"po{h}",
                                       tag="pp" if self.pool is ppsum else "po")
                        for h in range(hl)
                    ]
                bi, _ = self.w
                for kk in (2 * local, 2 * local + 1):
                    for h in range(hl):
                        nc.tensor.matmul(
                            self.po[h][:],
                            vh_sb[:, vidx(h, bi, kk), :],
                            et_rhs(self.ets[h][self.j][kk]),
                            start=(kk == 0),
                            stop=(kk == ST - 1),
                        )
                if local == 7:
                    bi, sqh = self.w
                    for h in range(hl):
                        ot = outpool.tile([DH + 1, 512], F32, name="ot", tag="ot")
                        # copy on the act engine (DVE is exp-loaded)
                        nc.scalar.copy(ot[:], self.po[h][:])
                        ch = sqh * 2 + self.j
                        # final window: sync/act queues drain faster at end
                        eng = (nc.sync if h == 0 else nc.scalar) \
                            if self.final else nc.gpsimd
                        eng.dma_start(out[h, bi, ch][:, :], ot[:])

        # ---------------- prologue ----------------
        # DVE: warm-up scratch + the ones column of the vh ring
        nc.vector.memset(warm[:], 0.125)
        nc.vector.memset(vh_sb[:, :, DH : DH + 1], 1.0)
        wires["vec"][0] += 0.8

        # sync queue: wk, then k0 (cols 0-512, 512-1024, then half B), then
        # q0 half B
        nc.sync.dma_start(wk_sb[:], wk[:, :]); wadd("sync", 0.26e6)
        k0t = [kpool.tile([P, s], BF16, name=f"k0_{kt}", tag="ks")
               for kt in range(KT)]
        q0t = [qpool.tile([P, s], BF16, name=f"q0_{kt}", tag="qs")
               for kt in range(KT)]
        v0t = [vpool.tile([P, s], BF16, name=f"v0_{kt}", tag="vs")
               for kt in range(KT)]
        kmark = []
        for blk in (0, 1):
            for kt in range(KT):
                nc.sync.dma_start(
                    k0t[kt][:, blk * 512 : (blk + 1) * 512],
                    kTd[0, 0, kt][:, blk * 512 : (blk + 1) * 512],
                )
            kmark.append(wadd("sync", 8 * P * 512 * 2, ndesc=8))
        for kt in range(KT):
            nc.sync.dma_start(k0t[kt][:, 1024:2048], kTd[0, 1, kt][:, :])
        m = wadd("sync", 8 * P * 1024 * 2, ndesc=8)
        kmark.extend([m, m])
        # act queue: wq, q0 cols 512-1024 (done well before the first exps)
        nc.scalar.dma_start(wq_sb[:], wq[:, :]); wadd("act", 0.26e6)
        for kt in range(KT):
            nc.scalar.dma_start(q0t[kt][:, 512:1024], qT[0, 0, kt][:, 512:1024])
        qmark1 = wadd("act", 8 * P * 512 * 2, ndesc=8)
        # gps queue: q0 cols 0-512 (ahead of wv/v0; v deadlines have slack)
        for kt in range(KT):
            nc.gpsimd.dma_start(q0t[kt][:, 0:512], qT[0, 0, kt][:, 0:512])
        qmark0 = wadd("gps", 8 * P * 512 * 2, ndesc=8)
        # sync queue: q0 half B (needed from window (0,1))
        for kt in range(KT):
            nc.sync.dma_start(q0t[kt][:, 1024:2048], qT[0, 1, kt][:, :])
        m = wadd("sync", 8 * P * 1024 * 2, ndesc=8)
        qmark23 = [m, m]
        # gps queue: wv, v0 as two contiguous half chunks per kt
        nc.gpsimd.dma_start(wv_sb[:], wv[:, :]); wadd("gps", 0.26e6)
        vmark = []
        for hblk in range(2):
            for kt in range(KT):
                nc.gpsimd.dma_start(
                    v0t[kt][:, hblk * 1024 : (hblk + 1) * 1024],
                    vT[0, hblk, kt][:, :],
                )
            m = wadd("gps", 8 * P * 1024 * 2, ndesc=8)
            vmark.extend([m, m])

        # PE warm-up against the memset tile: p-state ramp + fill bridge
        warmps = ppsum.tile([P, 512], F32, name="warm", tag="pp")
        for _ in range(8):
            nc.tensor.matmul(
                warmps[:], warm[:, 0:128], warm[:, 128:640], start=True, stop=True
            )
        # eager: k-proj blk0, q-proj blk0,1 (PE waits on their DMAs)
        qk_group(0, k0t, wk_sb, khT_sb, 0)
        qk_group(0, q0t, wq_sb, qhT_sb, 0)
        qk_group(0, q0t, wq_sb, qhT_sb, 1)

        # drip queue: (cost, min_step, deadline, fn)
        pending = deque()

        def mkitems_b0():
            items = []
            for blk in (1, 2, 3):
                items.append((4.0, min(rg(kmark[blk]), 4 * blk), 4 * blk,
                              lambda blk=blk: qk_group(0, k0t, wk_sb, khT_sb, blk)))
            for st in range(ST):
                gate = max(2 + st // 2, rg(vmark[st // 4]))
                items.append((1.0, min(gate, 9 + st // 2), 9 + st // 2,
                              lambda st=st: v_group(0, v0t, st)))
            for i, blk in enumerate((2, 3)):
                items.append((4.0, min(rg(qmark23[i]), ST), ST,
                              lambda blk=blk: qk_group(0, q0t, wq_sb, qhT_sb, blk)))
            items.sort(key=lambda it: it[1])
            return items

        pending.extend(mkitems_b0())

        # deferred stream descriptors: (min_step, fn) on sync / gps queues
        sync_descs = deque()
        gps_descs = deque()

        def emit_streams(nb, g_enq):
            """Allocate batch nb stream tiles; defer per-half chunk DMAs.
            k halves on sync (2/step from +9); q then v halves on gps
            (2/step from +12). Slot reuse WAR is schedule-guaranteed free
            by those steps. Per-half arrival gates let each window start on
            its first half while the second streams in."""
            kt_ = [kpool.tile([P, s], BF16, name=f"k{nb}_{kt}", tag="ks")
                   for kt in range(KT)]
            qt_ = [qpool.tile([P, s], BF16, name=f"q{nb}_{kt}", tag="qs")
                   for kt in range(KT)]
            vt_ = [vpool.tile([P, s], BF16, name=f"v{nb}_{kt}", tag="vs")
                   for kt in range(KT)]

            def chunk(eng, tiles, src, i, hf):
                eng.dma_start(tiles[i][:, hf * 1024 : (hf + 1) * 1024],
                              src[nb, hf, i][:, :])

            HB = KT * P * 1024 * 2  # bytes per half (2.1MB)
            marks = {}
            for name, queue, start, eng, tiles, src in (
                ("k", "sync", 9, nc.sync, kt_, kTd),
                ("q", "gps", 12, nc.gpsimd, qt_, qT),
                ("v", "gps", 20, nc.gpsimd, vt_, vT),
            ):
                dq = sync_descs if queue == "sync" else gps_descs
                for hf in range(2):
                    s0 = start + 4 * hf
                    for i in range(KT):
                        dq.append((g_enq + s0 + i // 2,
                                   lambda eng=eng, tiles=tiles, src=src, i=i,
                                   hf=hf: chunk(eng, tiles, src, i, hf)))
                    w = wires[queue]
                    w[0] = max(w[0], FILL_US + STEP_US * (g_enq + s0))
                    marks[(name, hf)] = rg(wadd(queue, HB, ndesc=KT))
            # proj items with deadline caps
            items = []
            base = g_enq + 2 * ST  # consumer window start
            for blk in range(NB):
                g = marks[("k", blk // 2)]
                items.append((4.0, min(g + blk, base + 4 * blk), base + 4 * blk,
                              lambda blk=blk: qk_group(nb, kt_, wk_sb, khT_sb, blk)))
            for blk in range(NB):
                dl = base + (0 if blk < 2 else ST)
                g = marks[("q", blk // 2)]
                items.append((4.0, min(g + blk, dl), dl,
                              lambda blk=blk: qk_group(nb, qt_, wq_sb, qhT_sb, blk)))
            for st in range(ST):
                dl = base + 9 + st // 2
                g = marks[("v", st // 8)]
                items.append((1.0, min(g + st // 2, dl), dl,
                              lambda st=st: v_group(nb, vt_, st)))
            items.sort(key=lambda it: it[1])
            return items

        # ---------------- main loop ----------------
        windows = [(bi, sqh) for bi in range(b) for sqh in range(NW)]
        LASTW = len(windows) - 1

        budget = 0.0
        j0_prev = None  # prev window's j0 pass: kk 14,15 + finalize at kt==0
        j1_prev = None  # prev window's j1 pass: runs at kt 1..8
        for w_idx, w in enumerate(windows):
            bi, sqh = w
            if sqh == 0 and bi + 1 < b:
                pending.extend(emit_streams(bi + 1, w_idx * ST))
            if w_idx == LASTW:
                assert not pending, (
                    f"pending proj items at last window: {len(pending)}"
                )
            ets = [[[], []], [[], []]]  # ets[h][j] -> list of 16 chunk tiles
            j0_cur = None
            j1_last = None
            for kt in range(ST):
                g = w_idx * ST + kt
                # deferred stream descriptors first (wire-critical, cheap)
                for _ in range(2):
                    if sync_descs and sync_descs[0][0] <= g:
                        sync_descs.popleft()[1]()
                for _ in range(2):
                    if gps_descs and gps_descs[0][0] <= g:
                        gps_descs.popleft()[1]()
                # drip: vh/qhT/khT writes must precede their readers
                budget = min(budget + 4.0, 6.0)
                while pending and pending[0][1] <= g and (
                        budget >= pending[0][0] or g >= pending[0][2]):
                    cost, _, _, fn = pending.popleft()
                    budget -= cost
                    fn()
                # PV before scores: reads only prior steps' et chunks, and the
                # finalize copies land ahead of this step's exps on act
                if kt == 0:
                    if j0_prev is not None:
                        j0_prev.emit(7)
                        j0_prev = None
                elif kt <= 8:
                    if j1_prev is not None:
                        j1_prev.emit(kt - 1)
                        if kt == 8:
                            j1_prev = None
                else:
                    if kt == 9:
                        j0_cur = PVPass(w, ets, 0, vpsum, final=(w_idx == LASTW))
                    j0_cur.emit(kt - 9)
                score_step(w, kt, ets)
                # last window: j1 in-window, lag-1 on odd kt (po in ppsum)
                if w_idx == LASTW and kt % 2 == 1:
                    if j1_last is None:
                        j1_last = PVPass(w, ets, 1, ppsum, final=True)
                    j1_last.emit((kt - 1) // 2)
            j0_prev = j0_cur
            if w_idx != LASTW:
                j1_prev = PVPass(w, ets, 1, vpsum)

        # epilogue: only w7's j0 finalize trails the last score step
        j0_prev.emit(7)
        assert not pending and not sync_descs and not gps_descs

    nc.compile()
    return nc


def _prep_inputs(q, k, v, Wq, Wk, Wv):
    """Host-side sharding + layout prep. Returns in_maps for 8 cores."""
    bf = ml_dtypes.bfloat16

    def xprep(x):
        # [B,S,D] -> chunk-major [B, half, kt, p, 1024]: each (b,half,kt)
        # chunk is a contiguous 256KB DRAM read for the kernel's stream DMAs
        xT = x.reshape(B * S, D).T.astype(bf)          # [D, B*S]
        xc = xT.reshape(8, 128, B, 2, 1024).transpose(2, 3, 0, 1, 4)
        return np.ascontiguousarray(xc)

    qT = xprep(q)
    kT = xprep(k)
    vT = xprep(v)
    scale = 1.0 / np.sqrt(DH)

    def wprep(w):
        # [d, FW] -> [p, kt*FW] so the kernel loads each weight with one DMA
        wt = w.T.reshape(8, 128, 128).transpose(1, 0, 2).reshape(128, 1024)
        return np.ascontiguousarray(wt).astype(bf)

    in_maps = []
    for c in range(N_CORES):
        rows = slice(c * HL * DH, (c + 1) * HL * DH)
        in_maps.append(
            {
                "qT": qT,
                "kT": kT,
                "vT": vT,
                "wq": wprep(Wq[rows, :] * scale),
                "wk": wprep(Wk[rows, :]),
                "wv": wprep(Wv[rows, :]),
            }
        )
    return in_maps


_NC_CACHE = {}


def _get_nc():
    if "nc" not in _NC_CACHE:
        _NC_CACHE["nc"] = build_attention_nc()
    return _NC_CACHE["nc"]


def kernel(q, k, v, attention_mask, Wq, bq, Wk, bk, Wv, bv, _trace=False):
    q = np.asarray(q, dtype=np.float32)
    k = np.asarray(k, dtype=np.float32)
    v = np.asarray(v, dtype=np.float32)
    Wq = np.asarray(Wq, dtype=np.float32)
    Wk = np.asarray(Wk, dtype=np.float32)
    Wv = np.asarray(Wv, dtype=np.float32)
    in_maps = _prep_inputs(q, k, v, Wq, Wk, Wv)
    nc = _get_nc()
    res = bass_utils.run_bass_kernel_spmd(
        nc, in_maps, core_ids=list(range(N_CORES)), trace=_trace
    )
    full = np.empty((B, S, D), dtype=np.float32)
    for c in range(N_CORES):
        # [HL, B, 4 chunks, 65, 512]
        o = np.asarray(res.results[c]["out"], dtype=np.float32)
        un = o[:, :, :, :DH, :]
        den = o[:, :, :, DH : DH + 1, :]
        norm = un / den  # [HL, B, 4, DH, 512]
        blk = np.transpose(norm, (1, 2, 4, 0, 3)).reshape(B, S, HL * DH)
        full[:, :, c * HL * DH : (c + 1) * HL * DH] = blk
    if _trace:
        kernel._last_exec_time_ns = res.exec_time_ns
        kernel._last_results = res
    return full


# revision 22
# speedup vs baseline: 1.0092x; 1.0092x over previous
"""Trainium2 Bass kernel for multi-head attention (B=4, S=2048, D=1024, H=16).

Sharding: tensor-parallel over heads. 8 cores x 2 heads each.
Each core receives the full (transposed, bf16) q/k/v and its own head-slice
of the projection weights; it computes its heads' attention and writes an
unnormalized output [h, b, 65, S] where row 64 is the softmax denominator.
Host divides and reassembles.

Schedule (v2 — restructured fill/steady/tail vs the first version):
 - Per-tensor DMA queues: k on SP(sync), q0 split over DVE/Act/SP queues,
   batch>=1 q and all v on the GpSimd queue, outputs on GpSimd. The Act and
   DVE queues carry no steady-state DMA, so exp consumers never stall
   behind descriptor issue.
 - Batch-0 streams land column-block-major (4x8 small DMAs per tensor) so
   the first projection group's inputs arrive ~2x earlier; the first score
   step fires at ~14us instead of ~25us.
 - PE warm-up runs against a memset tile (no DMA dependency), starting the
   p-state ramp at ~6.5us.
 - qhT/khT/vh are 2-batch ring buffers, paying for per-tensor stream pools
   (k/q/v x 8 bufs) whose slot-reuse WAR waits are schedule-guaranteed to
   be no-ops (no descriptor-queue stalls).
 - Projection work drips between score steps gated by a per-queue wire
   model, with hard deadline caps so a mis-estimated gate can only stall
   the PE, never reorder past a consumer.
 - The last window runs its j1 PV pass in-window (lag-1 on odd kt, po in
   the ppsum pool) so only one PV emit + copies trail the final score.

Per-core steady state: windows of (batch, 1024 q-cols) proceed in 16
kt-steps; each step emits 4 score matmuls into four single-bank [128,512]
PSUM chunks, 4 exps split between Act table-exp and DVE Schraudolph PWL
(int16 bf16-bit-patterns, f=0.5 uniform), and 4 PV matmuls (one pass of 2
po tiles [65,512] at a time, row 64 = softmax denominator via a ones
column in vh).

Math notes:
 - attention_mask is all-False in the problem spec -> no-op; biases zero.
 - 1/sqrt(d_head) folded into Wq on the host.
 - softmax without max-subtraction: scores ~ N(0,1), exp safe in fp32.
 - PWL exp on 1/2 of tiles adds ~1.2e-2 rel err (validated on HW:
   1.36e-2 total), under the 2e-2 gate with ~30% margin.
"""

import os
import sys

import numpy as np

try:
    import concourse.bass as bass
except ImportError:
    sys.path.insert(0, "/opt/trn_rl_repo")
    import concourse.bass as bass

import ml_dtypes
from collections import deque
from contextlib import ExitStack

import concourse.tile as tile
from concourse import bacc, mybir
from concourse import bass_utils

BF16 = mybir.dt.bfloat16
F32 = mybir.dt.float32
I16 = mybir.dt.int16

# Problem sizes (hardcoded per spec)
B = 4
S = 2048
D = 1024
H = 16
DH = 64
N_CORES = 8
HL = H // N_CORES  # heads per core = 2

# PWL exp: int16 = round(x * 128*log2(e) + (127*128 - c)); bits = bf16(~e^x)
PWL_A = 128.0 * 1.4426950408889634
PWL_B = 127.0 * 128.0 - 7.33


def build_attention_nc(b=B, s=S, d=D, hl=HL, num_devices=N_CORES):
    """Build the per-core Bass graph. Same graph on all cores (SPMD)."""
    P = 128  # partitions
    KT = d // P          # contraction tiles for projections = 8
    ST = s // P          # sk tiles per sequence = 16
    NB = s // 512        # 512-wide blocks per sequence = 4
    NW = s // 1024       # windows per batch = 2
    FW = hl * DH         # feature width this core computes (= 128)
    assert FW == 128 and s % 1024 == 0

    nc = bacc.Bacc(
        "TRN2",
        target_bir_lowering=False,
        debug=False,
        num_devices=num_devices,
    )

    # host ships q/k/v chunk-major: [b, half, kt, p, 1024] so every stream
    # DMA chunk is a 256KB contiguous DRAM read (128KB for 512-col subreads)
    qT = nc.dram_tensor("qT", [b, 2, d // 128, 128, 1024], BF16,
                        kind="ExternalInput").ap()
    kTd = nc.dram_tensor("kT", [b, 2, d // 128, 128, 1024], BF16,
                         kind="ExternalInput").ap()
    vT = nc.dram_tensor("vT", [b, 2, d // 128, 128, 1024], BF16,
                        kind="ExternalInput").ap()
    # weights arrive host-permuted as [p, kt*FW] so one DMA loads each
    wq = nc.dram_tensor("wq", [P, KT * FW], BF16, kind="ExternalInput").ap()
    wk = nc.dram_tensor("wk", [P, KT * FW], BF16, kind="ExternalInput").ap()
    wv = nc.dram_tensor("wv", [P, KT * FW], BF16, kind="ExternalInput").ap()
    # output as contiguous [65, 512] chunks (chunk = sqh*2+j) so each out DMA
    # is a single large-run transfer: fast completion posting, cheap drain
    out = nc.dram_tensor(
        "out", [hl, b, (s // 1024) * 2, DH + 1, 512], F32, kind="ExternalOutput"
    ).ap()

    with tile.TileContext(nc) as tc, ExitStack() as ctx:
        persist = ctx.enter_context(tc.tile_pool(name="persist", bufs=1))
        # one stream pool; slots are per-tag (ks/qs/vs x KT each), so batch
        # n+1's tile i reuses batch n's slot i — released a full window
        # earlier by schedule. One pool = one release barrier at teardown.
        xpool = ctx.enter_context(tc.tile_pool(name="xpool", bufs=KT))
        kpool = qpool = vpool = xpool
        spsum = ctx.enter_context(tc.tile_pool(name="spsum", bufs=4, space="PSUM"))
        ppsum = ctx.enter_context(tc.tile_pool(name="ppsum", bufs=2, space="PSUM"))
        vpsum = ctx.enter_context(tc.tile_pool(name="vpsum", bufs=2, space="PSUM"))
        epool = ctx.enter_context(tc.tile_pool(name="epool", bufs=64))
        outpool = ctx.enter_context(tc.tile_pool(name="outpool", bufs=4))

        # weights in SBUF: [128, KT*128], k-tile kt at cols kt*128:(kt+1)*128
        wq_sb = persist.tile([P, KT * FW], BF16, tag="wq_sb")
        wk_sb = persist.tile([P, KT * FW], BF16, tag="wk_sb")
        wv_sb = persist.tile([P, KT * FW], BF16, tag="wv_sb")

        # projected activations: 2-batch ring buffers
        qhT_sb = persist.tile([P, 2 * s], BF16, tag="qhT_sb")
        khT_sb = persist.tile([P, 2 * s], BF16, tag="khT_sb")
        # vh ring: per (h, bi%2, st): [128, 65] block, col 64 == 1.0
        vh_sb = persist.tile([P, hl * 2 * ST, DH + 1], BF16, tag="vh_sb")
        # warm-up scratch (memset, no DMA dependency)
        warm = persist.tile([P, 640], BF16, tag="warm")

        def vidx(h, bi, st):
            return (h * 2 + bi % 2) * ST + st

        # ---------------- wire model (per DMA queue, in us) ----------------
        QSTART = 6.3     # engines free after framework preamble
        DESC_US = 0.68   # per-descriptor issue cost on the queue engine
        WIRE_BPUS = 220e3  # bytes per us per queue
        FILL_US = 12.0   # est. wall time of global step 0 (conservative low)
        STEP_US = 1.9    # optimistic step period (conservative for gating)
        wires = {q: [QSTART, 0] for q in ("sync", "vec", "act", "gps")}

        def wadd(q, nbytes, ndesc=1):
            w = wires[q]
            w[1] += ndesc
            w[0] = max(w[0] + nbytes / WIRE_BPUS,
                       QSTART + DESC_US * w[1] + nbytes / WIRE_BPUS / max(ndesc, 1))
            return w[0]

        def rg(t_us, margin=1):
            return int(np.ceil((t_us - FILL_US) / STEP_US)) + margin

        ENG = {"sync": None, "vec": None, "act": None, "gps": None}

        def qeng(q):
            return {"sync": nc.sync, "vec": nc.vector, "act": nc.scalar,
                    "gps": nc.gpsimd}[q]

        # ---------------- compute groups ----------------
        def qk_group(bi, xs_t, w_sb, dst, blk):
            """One q/k projection block group: 8 MMs + 1 cast copy."""
            ps = ppsum.tile([P, 512], F32, name="projp", tag="pp")
            for kt in range(KT):
                nc.tensor.matmul(
                    ps[:],
                    w_sb[:, kt * FW : (kt + 1) * FW],
                    xs_t[kt][:, blk * 512 : (blk + 1) * 512],
                    start=(kt == 0),
                    stop=(kt == KT - 1),
                )
            base = (bi % 2) * s
            nc.scalar.copy(dst[:, base + blk * 512 : base + (blk + 1) * 512], ps[:])

        def v_group(bi, xs_t, st):
            """One v projection st group: 8 MMs + 2 head copies into vh."""
            pv = ppsum.tile([P, FW], F32, name="vproj", tag="pp")
            for kt in range(KT):
                nc.tensor.matmul(
                    pv[:],
                    xs_t[kt][:, st * P : (st + 1) * P],
                    wv_sb[:, kt * FW : (kt + 1) * FW],
                    start=(kt == 0),
                    stop=(kt == KT - 1),
                )
            for h in range(hl):
                nc.vector.tensor_copy(
                    vh_sb[:, vidx(h, bi, st), 0:DH], pv[:, h * DH : (h + 1) * DH]
                )

        def et_rhs(ent):
            t, is_i16 = ent
            return t[:].bitcast(BF16) if is_i16 else t[:]

        def score_step(w, kt, ets):
            """4 score chunk MMs (head pairs pack) + 4 per-chunk exps."""
            bi, sqh = w
            q0 = (bi % 2) * s + sqh * 1024
            k0 = (bi % 2) * s + kt * P
            cs = {}
            for j in range(2):
                for h in range(hl):
                    hp = h * DH
                    c = spsum.tile([P, 512], F32, name=f"c{h}{j}", tag="sc")
                    nc.tensor.matmul(
                        c[:],
                        khT_sb[hp : hp + DH, k0 : k0 + P],
                        qhT_sb[hp : hp + DH, q0 + j * 512 : q0 + (j + 1) * 512],
                        start=True,
                        stop=True,
                    )
                    cs[(h, j)] = c
            for j in range(2):
                for h in range(hl):
                    # alternate act/DVE per chunk: f=0.5 per head, uniform
                    use_dve = (kt + h + j) % 2 == 0
                    if use_dve:
                        t = epool.tile([P, 512], I16, name="etd", tag="et")
                        nc.vector.tensor_scalar(
                            t[:], cs[(h, j)][:], PWL_A, PWL_B,
                            mybir.AluOpType.mult, mybir.AluOpType.add,
                        )
                        ets[h][j].append((t, True))
                    else:
                        t = epool.tile([P, 512], BF16, name="eta", tag="et")
                        nc.scalar.activation(
                            t[:], cs[(h, j)][:], mybir.ActivationFunctionType.Exp
                        )
                        ets[h][j].append((t, False))

        class PVPass:
            """One PV pass (w, j): 2 po tiles accumulated over 16 kk."""

            def __init__(self, w, ets, j, pool, final=False):
                self.w = w
                self.ets = ets
                self.j = j
                self.pool = pool
                self.final = final
                self.po = None

            def emit(self, local):
                """local in 0..7 -> kk = 2*local, 2*local+1 (4 MMs)."""
                if local == 0:
                    self.po = [
                        self.pool.tile([DH + 1, 512], F32, name=f"po{h}",
                                       tag="pp" if self.pool is ppsum else "po")
                        for h in range(hl)
                    ]
                bi, _ = self.w
                for kk in (2 * local, 2 * local + 1):
                    for h in range(hl):
                        nc.tensor.matmul(
                            self.po[h][:],
                            vh_sb[:, vidx(h, bi, kk), :],
                            et_rhs(self.ets[h][self.j][kk]),
                            start=(kk == 0),
                            stop=(kk == ST - 1),
                        )
                if local == 7:
                    bi, sqh = self.w
                    for h in range(hl):
                        ot = outpool.tile([DH + 1, 512], F32, name="ot", tag="ot")
                        # copies on act (DVE is exp-loaded); final window
                        # splits across act+DVE so the two heads drain in
                        # parallel right after the last score step
                        ceng = (nc.scalar if h == 0 else nc.vector) \
                            if self.final else nc.scalar
                        (ceng.copy if ceng is nc.scalar
                         else ceng.tensor_copy)(ot[:], self.po[h][:])
                        ch = sqh * 2 + self.j
                        # final window: sync/act queues drain faster at end
                        eng = (nc.sync if h == 0 else nc.scalar) \
                            if self.final else nc.gpsimd
                        eng.dma_start(out[h, bi, ch][:, :], ot[:])

        # ---------------- prologue ----------------
        # DVE: warm-up scratch + the ones column of the vh ring
        nc.vector.memset(warm[:], 0.125)
        nc.vector.memset(vh_sb[:, :, DH : DH + 1], 1.0)
        wires["vec"][0] += 0.8

        # sync queue: wk, then k0 (cols 0-512, 512-1024, then half B), then
        # q0 half B
        nc.sync.dma_start(wk_sb[:], wk[:, :]); wadd("sync", 0.26e6)
        k0t = [kpool.tile([P, s], BF16, name=f"k0_{kt}", tag="ks")
               for kt in range(KT)]
        q0t = [qpool.tile([P, s], BF16, name=f"q0_{kt}", tag="qs")
               for kt in range(KT)]
        v0t = [vpool.tile([P, s], BF16, name=f"v0_{kt}", tag="vs")
               for kt in range(KT)]
        kmark = []
        for blk in (0, 1):
            for kt in range(KT):
                nc.sync.dma_start(
                    k0t[kt][:, blk * 512 : (blk + 1) * 512],
                    kTd[0, 0, kt][:, blk * 512 : (blk + 1) * 512],
                )
            kmark.append(wadd("sync", 8 * P * 512 * 2, ndesc=8))
        for kt in range(KT):
            nc.sync.dma_start(k0t[kt][:, 1024:2048], kTd[0, 1, kt][:, :])
        m = wadd("sync", 8 * P * 1024 * 2, ndesc=8)
        kmark.extend([m, m])
        # act queue: wq, q0 cols 512-1024 (done well before the first exps)
        nc.scalar.dma_start(wq_sb[:], wq[:, :]); wadd("act", 0.26e6)
        for kt in range(KT):
            nc.scalar.dma_start(q0t[kt][:, 512:1024], qT[0, 0, kt][:, 512:1024])
        qmark1 = wadd("act", 8 * P * 512 * 2, ndesc=8)
        # gps queue: q0 cols 0-512 (ahead of wv/v0; v deadlines have slack)
        for kt in range(KT):
            nc.gpsimd.dma_start(q0t[kt][:, 0:512], qT[0, 0, kt][:, 0:512])
        qmark0 = wadd("gps", 8 * P * 512 * 2, ndesc=8)
        # sync queue: q0 half B (needed from window (0,1))
        for kt in range(KT):
            nc.sync.dma_start(q0t[kt][:, 1024:2048], qT[0, 1, kt][:, :])
        m = wadd("sync", 8 * P * 1024 * 2, ndesc=8)
        qmark23 = [m, m]
        # gps queue: wv, v0 as two contiguous half chunks per kt
        nc.gpsimd.dma_start(wv_sb[:], wv[:, :]); wadd("gps", 0.26e6)
        vmark = []
        for hblk in range(2):
            for kt in range(KT):
                nc.gpsimd.dma_start(
                    v0t[kt][:, hblk * 1024 : (hblk + 1) * 1024],
                    vT[0, hblk, kt][:, :],
                )
            m = wadd("gps", 8 * P * 1024 * 2, ndesc=8)
            vmark.extend([m, m])

        # PE warm-up against the memset tile: p-state ramp + fill bridge
        warmps = ppsum.tile([P, 512], F32, name="warm", tag="pp")
        for _ in range(8):
            nc.tensor.matmul(
                warmps[:], warm[:, 0:128], warm[:, 128:640], start=True, stop=True
            )
        # eager: k-proj blk0, q-proj blk0,1 (PE waits on their DMAs)
        qk_group(0, k0t, wk_sb, khT_sb, 0)
        qk_group(0, q0t, wq_sb, qhT_sb, 0)
        qk_group(0, q0t, wq_sb, qhT_sb, 1)

        # drip queue: (cost, min_step, deadline, fn)
        pending = deque()

        def mkitems_b0():
            items = []
            for blk in (1, 2, 3):
                items.append((4.0, min(rg(kmark[blk]), 4 * blk), 4 * blk,
                              lambda blk=blk: qk_group(0, k0t, wk_sb, khT_sb, blk)))
            for st in range(ST):
                gate = max(2 + st // 2, rg(vmark[st // 4]))
                items.append((1.0, min(gate, 9 + st // 2), 9 + st // 2,
                              lambda st=st: v_group(0, v0t, st)))
            for i, blk in enumerate((2, 3)):
                items.append((4.0, min(rg(qmark23[i]), ST), ST,
                              lambda blk=blk: qk_group(0, q0t, wq_sb, qhT_sb, blk)))
            items.sort(key=lambda it: it[1])
            return items

        pending.extend(mkitems_b0())

        # deferred stream descriptors: (min_step, fn) on sync / gps queues
        sync_descs = deque()
        gps_descs = deque()

        def emit_streams(nb, g_enq):
            """Allocate batch nb stream tiles; defer per-half chunk DMAs.
            k halves on sync (2/step from +9); q then v halves on gps
            (2/step from +12). Slot reuse WAR is schedule-guaranteed free
            by those steps. Per-half arrival gates let each window start on
            its first half while the second streams in."""
            kt_ = [kpool.tile([P, s], BF16, name=f"k{nb}_{kt}", tag="ks")
                   for kt in range(KT)]
            qt_ = [qpool.tile([P, s], BF16, name=f"q{nb}_{kt}", tag="qs")
                   for kt in range(KT)]
            vt_ = [vpool.tile([P, s], BF16, name=f"v{nb}_{kt}", tag="vs")
                   for kt in range(KT)]

            def chunk(eng, tiles, src, i, hf):
                eng.dma_start(tiles[i][:, hf * 1024 : (hf + 1) * 1024],
                              src[nb, hf, i][:, :])

            HB = KT * P * 1024 * 2  # bytes per half (2.1MB)
            marks = {}
            for name, queue, start, eng, tiles, src in (
                ("k", "sync", 9, nc.sync, kt_, kTd),
                ("q", "gps", 12, nc.gpsimd, qt_, qT),
                ("v", "gps", 20, nc.gpsimd, vt_, vT),
            ):
                dq = sync_descs if queue == "sync" else gps_descs
                for hf in range(2):
                    s0 = start + 4 * hf
                    for i in range(KT):
                        dq.append((g_enq + s0 + i // 2,
                                   lambda eng=eng, tiles=tiles, src=src, i=i,
                                   hf=hf: chunk(eng, tiles, src, i, hf)))
                    w = wires[queue]
                    w[0] = max(w[0], FILL_US + STEP_US * (g_enq + s0))
                    marks[(name, hf)] = rg(wadd(queue, HB, ndesc=KT))
            # proj items with deadline caps
            items = []
            base = g_enq + 2 * ST  # consumer window start
            for blk in range(NB):
                g = marks[("k", blk // 2)]
                items.append((4.0, min(g + blk, base + 4 * blk), base + 4 * blk,
                              lambda blk=blk: qk_group(nb, kt_, wk_sb, khT_sb, blk)))
            for blk in range(NB):
                dl = base + (0 if blk < 2 else ST)
                g = marks[("q", blk // 2)]
                items.append((4.0, min(g + blk, dl), dl,
                              lambda blk=blk: qk_group(nb, qt_, wq_sb, qhT_sb, blk)))
            for st in range(ST):
                dl = base + 9 + st // 2
                g = marks[("v", st // 8)]
                items.append((1.0, min(g + st // 2, dl), dl,
                              lambda st=st: v_group(nb, vt_, st)))
            items.sort(key=lambda it: it[1])
            return items

        # ---------------- main loop ----------------
        windows = [(bi, sqh) for bi in range(b) for sqh in range(NW)]
        LASTW = len(windows) - 1

        budget = 0.0
        j0_prev = None  # prev window's j0 pass: kk 14,15 + finalize at kt==0
        j1_prev = None  # prev window's j1 pass: runs at kt 1..8
        for w_idx, w in enumerate(windows):
            bi, sqh = w
            if sqh == 0 and bi + 1 < b:
                pending.extend(emit_streams(bi + 1, w_idx * ST))
            if w_idx == LASTW:
                assert not pending, (
                    f"pending proj items at last window: {len(pending)}"
                )
            ets = [[[], []], [[], []]]  # ets[h][j] -> list of 16 chunk tiles
            j0_cur = None
            j1_last = None
            for kt in range(ST):
                g = w_idx * ST + kt
                # deferred stream descriptors first (wire-critical, cheap)
                for _ in range(2):
                    if sync_descs and sync_descs[0][0] <= g:
                        sync_descs.popleft()[1]()
                for _ in range(2):
                    if gps_descs and gps_descs[0][0] <= g:
                        gps_descs.popleft()[1]()
                # drip: vh/qhT/khT writes must precede their readers
                budget = min(budget + 4.0, 6.0)
                while pending and pending[0][1] <= g and (
                        budget >= pending[0][0] or g >= pending[0][2]):
                    cost, _, _, fn = pending.popleft()
                    budget -= cost
                    fn()
                # PV before scores: reads only prior steps' et chunks, and the
                # finalize copies land ahead of this step's exps on act
                if kt == 0:
                    if j0_prev is not None:
                        j0_prev.emit(7)
                        j0_prev = None
                elif kt <= 8:
                    if j1_prev is not None:
                        j1_prev.emit(kt - 1)
                        if kt == 8:
                            j1_prev = None
                else:
                    if kt == 9:
                        j0_cur = PVPass(w, ets, 0, vpsum, final=(w_idx == LASTW))
                    j0_cur.emit(kt - 9)
                score_step(w, kt, ets)
                # last window: j1 in-window, lag-1 on odd kt (po in ppsum)
                if w_idx == LASTW and kt % 2 == 1:
                    if j1_last is None:
                        j1_last = PVPass(w, ets, 1, ppsum, final=True)
                    j1_last.emit((kt - 1) // 2)
            j0_prev = j0_cur
            if w_idx != LASTW:
                j1_prev = PVPass(w, ets, 1, vpsum)

        # epilogue: only w7's j0 finalize trails the last score step
        j0_prev.emit(7)
        assert not pending and not sync_descs and not gps_descs

    nc.compile()
    return nc


def _prep_inputs(q, k, v, Wq, Wk, Wv):
    """Host-side sharding + layout prep. Returns in_maps for 8 cores."""
    bf = ml_dtypes.bfloat16

    def xprep(x):
        # [B,S,D] -> chunk-major [B, half, kt, p, 1024]: each (b,half,kt)
        # chunk is a contiguous 256KB DRAM read for the kernel's stream DMAs
        xT = x.reshape(B * S, D).T.astype(bf)          # [D, B*S]
        xc = xT.reshape(8, 128, B, 2, 1024).transpose(2, 3, 0, 1, 4)
        return np.ascontiguousarray(xc)

    qT = xprep(q)
    kT = xprep(k)
    vT = xprep(v)
    scale = 1.0 / np.sqrt(DH)

    def wprep(w):
        # [d, FW] -> [p, kt*FW] so the kernel loads each weight with one DMA
        wt = w.T.reshape(8, 128, 128).transpose(1, 0, 2).reshape(128, 1024)
        return np.ascontiguousarray(wt).astype(bf)

    in_maps = []
    for c in range(N_CORES):
        rows = slice(c * HL * DH, (c + 1) * HL * DH)
        in_maps.append(
            {
                "qT": qT,
                "kT": kT,
                "vT": vT,
                "wq": wprep(Wq[rows, :] * scale),
                "wk": wprep(Wk[rows, :]),
                "wv": wprep(Wv[rows, :]),
            }
        )
    return in_maps


_NC_CACHE = {}


def _get_nc():
    if "nc" not in _NC_CACHE:
        _NC_CACHE["nc"] = build_attention_nc()
    return _NC_CACHE["nc"]


def kernel(q, k, v, attention_mask, Wq, bq, Wk, bk, Wv, bv, _trace=False):
    q = np.asarray(q, dtype=np.float32)
    k = np.asarray(k, dtype=np.float32)
    v = np.asarray(v, dtype=np.float32)
    Wq = np.asarray(Wq, dtype=np.float32)
    Wk = np.asarray(Wk, dtype=np.float32)
    Wv = np.asarray(Wv, dtype=np.float32)
    in_maps = _prep_inputs(q, k, v, Wq, Wk, Wv)
    nc = _get_nc()
    res = bass_utils.run_bass_kernel_spmd(
        nc, in_maps, core_ids=list(range(N_CORES)), trace=_trace
    )
    full = np.empty((B, S, D), dtype=np.float32)
    for c in range(N_CORES):
        # [HL, B, 4 chunks, 65, 512]
        o = np.asarray(res.results[c]["out"], dtype=np.float32)
        un = o[:, :, :, :DH, :]
        den = o[:, :, :, DH : DH + 1, :]
        norm = un / den  # [HL, B, 4, DH, 512]
        blk = np.transpose(norm, (1, 2, 4, 0, 3)).reshape(B, S, HL * DH)
        full[:, :, c * HL * DH : (c + 1) * HL * DH] = blk
    if _trace:
        kernel._last_exec_time_ns = res.exec_time_ns
        kernel._last_results = res
    return full
